# revision 2
# baseline (speedup 1.0000x reference)
import sys

if "/opt/trn_rl_repo" not in sys.path:
    sys.path.insert(0, "/opt/trn_rl_repo")

import os
import numpy as np
from contextlib import ExitStack

import ml_dtypes

import concourse.tile as tile
from concourse import bacc, mybir
from concourse import bass_utils

F32 = mybir.dt.float32
BF16 = mybir.dt.bfloat16
AF = mybir.ActivationFunctionType
ALU = mybir.AluOpType
AX = mybir.AxisListType

B, C, L = 32, 128, 8192
N_CORES = 8
NB = B // N_CORES          # batches per core
CQ = C // 4
EPS = 1e-5
CH = 2048                  # abs / DMA chunk
PCH = 1024                 # p2 chunk (2 PSUM banks fp32)
OT = 512                   # p3 out tile (1 PSUM bank)

_BUILD_CACHE = {}


def _build(reps=1, loop_reps=0):
    key = (reps, loop_reps)
    if key in _BUILD_CACHE:
        return _BUILD_CACHE[key]

    nc = bacc.Bacc("TRN2", target_bir_lowering=False, debug=False)

    x_ap = nc.dram_tensor("x_dram", [NB, C, L], BF16, kind="ExternalInput").ap()
    w_v_ap = nc.dram_tensor("w_v", [C, C], BF16, kind="ExternalInput").ap()
    wsc0_ap = nc.dram_tensor("wsc0", [C, C], BF16, kind="ExternalInput").ap()
    wsc1_ap = nc.dram_tensor("wsc1", [C, C], BF16, kind="ExternalInput").ap()
    wsc2_ap = nc.dram_tensor("wsc2", [C, C], BF16, kind="ExternalInput").ap()
    w2t_ap = nc.dram_tensor("w2t", [C, C], BF16, kind="ExternalInput").ap()
    wfc1_ap = nc.dram_tensor("wfc1", [C, CQ], F32, kind="ExternalInput").ap()
    b1e_ap = nc.dram_tensor("b1e", [CQ, 1], F32, kind="ExternalInput").ap()
    wfc2_ap = nc.dram_tensor("wfc2", [CQ, C], F32, kind="ExternalInput").ap()
    b2_ap = nc.dram_tensor("b2", [C, 1], F32, kind="ExternalInput").ap()
    t2_ap = nc.dram_tensor("t2", [C, 1], F32, kind="ExternalInput").ap()
    wam_ap = nc.dram_tensor("wam", [C, C], F32, kind="ExternalInput").ap()
    wax_ap = nc.dram_tensor("wax", [C, C], F32, kind="ExternalInput").ap()
    ident_ap = nc.dram_tensor("ident", [C, C], F32, kind="ExternalInput").ap()
    out_ap = nc.dram_tensor("out_dram", [NB, C, L], BF16, kind="ExternalOutput").ap()
    debug = os.environ.get("K_DEBUG", "0") == "1"
    if debug:
        dbg_xr_ap = nc.dram_tensor("dbg_xr", [NB, C, 8], BF16, kind="ExternalOutput").ap()
        dbg_sf_ap = nc.dram_tensor("dbg_sf", [NB, C, 8], BF16, kind="ExternalOutput").ap()
        dbg_w2a_ap = nc.dram_tensor("dbg_w2a", [NB, C, C], BF16, kind="ExternalOutput").ap()
        dbg_st_ap = nc.dram_tensor("dbg_st", [NB, C, 4], F32, kind="ExternalOutput").ap()

    clamp_eng = os.environ.get("K_CLAMPENG", "vector")
    abs_mode = os.environ.get("K_ABSMODE", "absmax")
    out_eng = os.environ.get("K_OUTENG", "sync")
    dma_ch = int(os.environ.get("K_DMACH", "4096"))
    ost = int(os.environ.get("K_OST", "2048"))   # output stage width

    with tile.TileContext(nc) as tc, ExitStack() as ctx:
        wpool = ctx.enter_context(tc.tile_pool(name="wpool", bufs=1))
        xr_pool = ctx.enter_context(tc.tile_pool(name="xr", bufs=3))
        s_pool = ctx.enter_context(tc.tile_pool(name="soft", bufs=2))
        scr_pool = ctx.enter_context(tc.tile_pool(name="scr", bufs=2))
        vc_pool = ctx.enter_context(tc.tile_pool(name="vc", bufs=3))
        c_pool = ctx.enter_context(tc.tile_pool(name="cch", bufs=3))
        x1_pool = ctx.enter_context(tc.tile_pool(name="x1ch", bufs=3))
        out_pool = ctx.enter_context(tc.tile_pool(name="ot", bufs=3))
        st_pool = ctx.enter_context(tc.tile_pool(name="stats", bufs=2))
        row_pool = ctx.enter_context(tc.tile_pool(name="rows", bufs=2))
        w2a_pool = ctx.enter_context(tc.tile_pool(name="w2a", bufs=2))
        v_psp = ctx.enter_context(tc.tile_pool(name="v_ps", bufs=2, space="PSUM"))
        o_psp = ctx.enter_context(
            tc.tile_pool(name="o_ps", bufs=int(os.environ.get("K_OBUFS", "3")),
                         space="PSUM"))
        s_psp = ctx.enter_context(tc.tile_pool(name="s_ps", bufs=1, space="PSUM"))

        # ---- load weights (once) ----
        def wload(nm, ap, shape, dt=F32):
            t = wpool.tile(shape, dt, tag=nm)
            nc.sync.dma_start(t[:], ap[:])
            return t

        w_v_t = wload("w_v_t", w_v_ap, [C, C], BF16)
        wsc0_t = wload("wsc0_t", wsc0_ap, [C, C], BF16)
        wsc1_t = wload("wsc1_t", wsc1_ap, [C, C], BF16)
        wsc2_t = wload("wsc2_t", wsc2_ap, [C, C], BF16)
        w2t_t = wload("w2t_t", w2t_ap, [C, C], BF16)
        wfc1_t = wload("wfc1_t", wfc1_ap, [C, CQ])
        b1e_t = wload("b1e_t", b1e_ap, [CQ, 1])
        wfc2_t = wload("wfc2_t", wfc2_ap, [CQ, C])
        b2_t = wload("b2_t", b2_ap, [C, 1])
        t2_t = wload("t2_t", t2_ap, [C, 1])
        wam_t = wload("wam_t", wam_ap, [C, CQ * 0 + C])
        wax_t = wload("wax_t", wax_ap, [C, C])
        ident_t = wload("ident_t", ident_ap, [C, C])
        ones_t = wpool.tile([1, C], F32, tag="ones_t")
        nc.vector.memset(ones_t[:], 1.0)

        # ---- pipeline stages ----
        def p1_dma(b, st):
            xr = xr_pool.tile([C, L + 2], BF16, tag="xr")
            st["xr"] = xr
            nc.vector.memset(xr[:, 0:1], 0.0)
            nc.vector.memset(xr[:, L + 1:L + 2], 0.0)
            for q in range(L // dma_ch):
                nc.sync.dma_start(xr[:, 1 + q * dma_ch:1 + (q + 1) * dma_ch],
                                  x_ap[b, :, q * dma_ch:(q + 1) * dma_ch])

        def p1_abs(b, st):
            # sum|x| per channel. 3 chunks on Act (|x| via Abs+accum, the
            # baseline-proven accumulator path), 1 on DVE as a
            # sum-max(x,0)/sum-min(x,0) pair with fp32 primary outputs
            # (2x_2p mode; packed bf16-out accumulating ops misbehave on HW).
            xr = st["xr"]
            nq = L // CH
            nact = nq if abs_mode == "act" else 3 * nq // 4
            pos_p = st_pool.tile([C, nq], F32, tag="pos_p")
            neg_p = st_pool.tile([C, nq], F32, tag="neg_p")
            st["pos_p"], st["neg_p"] = pos_p, neg_p
            nc.vector.memset(neg_p[:, 0:nact], 0.0)
            for q in range(nq):
                xsl = xr[:, 1 + q * CH:1 + (q + 1) * CH]
                if q < nact:
                    scr = scr_pool.tile([C, CH], BF16, tag="scr")
                    nc.scalar.activation(scr[:], xsl, AF.Abs,
                                         accum_out=pos_p[:, q:q + 1])
                else:
                    scr = scr_pool.tile([C, CH], F32, tag="scr32", name="scr32")
                    nc.vector.tensor_scalar(scr[:], xsl, 0.0, None, ALU.max,
                                            ALU.add, accum_out=pos_p[:, q:q + 1])
                    scrn = scr_pool.tile([C, CH], F32, tag="scrn32", name="scrn32")
                    nc.vector.tensor_scalar(scrn[:], xsl, 0.0, None, ALU.min,
                                            ALU.add, accum_out=neg_p[:, q:q + 1])

        def mlp(b, st):
            spos = st_pool.tile([C, 1], F32, tag="spos")
            nc.vector.tensor_reduce(spos[:], st["pos_p"][:], AX.X, ALU.add)
            sneg = st_pool.tile([C, 1], F32, tag="sneg")
            nc.vector.tensor_reduce(sneg[:], st["neg_p"][:], AX.X, ALU.add)
            sabs = st_pool.tile([C, 1], F32, tag="sabs")
            nc.vector.tensor_tensor(sabs[:], spos[:], sneg[:], ALU.subtract)
            h_ps = s_psp.tile([CQ, 1], F32, tag="s_ps")
            nc.tensor.matmul(h_ps[:], wfc1_t[:], sabs[:], start=True, stop=True)
            h_t = st_pool.tile([CQ, 1], F32, tag="h_t")
            nc.scalar.activation(h_t[:], h_ps[:], AF.Relu, bias=b1e_t[:], scale=1.0)
            y_ps = s_psp.tile([C, 1], F32, tag="s_ps")
            nc.tensor.matmul(y_ps[:], wfc2_t[:], h_t[:], start=True, stop=True)
            x12 = st_pool.tile([C, 1], F32, tag="x12")
            nc.scalar.activation(x12[:], y_ps[:], AF.Sigmoid, bias=b2_t[:], scale=1.0)
            tpos = st_pool.tile([C, 1], F32, tag="tpos")
            nc.vector.scalar_tensor_tensor(tpos[:], sabs[:], 1.0 / L, x12[:],
                                           ALU.mult, ALU.mult)
            negt = st_pool.tile([C, 1], F32, tag="negt")
            nc.vector.scalar_tensor_tensor(negt[:], sabs[:], -1.0 / L, x12[:],
                                           ALU.mult, ALU.mult)
            st["tpos"], st["negt"] = tpos, negt

        def p2_mm(b, st, p):
            # v = w1 . x  for chunk p -> PSUM
            xr = st["xr"]
            v_ps = v_psp.tile([C, PCH], F32, tag="v_ps")
            base = 1 + p * PCH
            for j in range(PCH // 512):
                nc.tensor.matmul(v_ps[:, j * 512:(j + 1) * 512], w_v_t[:],
                                 xr[:, base + j * 512:base + (j + 1) * 512],
                                 start=True, stop=True)
            st[("v_ps", p)] = v_ps

        def p2_ew(b, st, p):
            # soft = v - clamp(v, -T, T); x1 = x + soft (stats only)
            xr, tpos, negt = st["xr"], st["tpos"], st["negt"]
            v_ps = st.pop(("v_ps", p))
            if p == 0:
                st["soft"] = s_pool.tile([C, L], BF16, tag="soft", name="soft")
                st["ssum_p"] = st_pool.tile([C, L // PCH], F32, tag="ssum_p", name="ssum_p")
                st["smax_p"] = st_pool.tile([C, L // PCH], F32, tag="smax_p", name="smax_p")
            soft, ssum_p, smax_p = st["soft"], st["ssum_p"], st["smax_p"]
            vc = vc_pool.tile([C, PCH], BF16, tag="vc")
            nc.scalar.activation(vc[:], v_ps[:], AF.Copy)
            cch = c_pool.tile([C, PCH], BF16, tag="cch")
            ceng = nc.gpsimd if clamp_eng == "gpsimd" else nc.vector
            ceng.tensor_scalar(cch[:], vc[:], tpos[:], negt[:], ALU.min, ALU.max)
            ssl = soft[:, p * PCH:(p + 1) * PCH]
            nc.vector.tensor_tensor(ssl, vc[:], cch[:], ALU.subtract)
            # x1 = soft + x with fused sum-accum (stt has no fast mode but
            # carries the baseline-proven accumulator path); fp32 out so the
            # following max-accum runs in safe 2x_2p mode
            x1c = x1_pool.tile([C, PCH], F32, tag="x1c")
            nc.vector.scalar_tensor_tensor(x1c[:], ssl, 0.0,
                                           xr[:, 1 + p * PCH:1 + (p + 1) * PCH],
                                           ALU.add, ALU.add,
                                           accum_out=ssum_p[:, p:p + 1])
            scr2 = scr_pool.tile([C, PCH], F32, tag="scr3", name="scr3")
            nc.vector.tensor_scalar(scr2[:], x1c[:], 0.0, None, ALU.add, ALU.max,
                                    accum_out=smax_p[:, p:p + 1])

        def ach_a(b, st):
            # stat reduces + conv-logit matmuls
            s_x1 = st_pool.tile([C, 1], F32, tag="s_x1")
            nc.vector.tensor_reduce(s_x1[:], st["ssum_p"][:], AX.X, ALU.add)
            mx = st_pool.tile([C, 1], F32, tag="mx")
            nc.vector.tensor_reduce(mx[:], st["smax_p"][:], AX.X, ALU.max)
            lg_ps = s_psp.tile([C, 1], F32, tag="s_ps")
            nc.tensor.matmul(lg_ps[:], wam_t[:], s_x1[:], start=True, stop=False)
            nc.tensor.matmul(lg_ps[:], wax_t[:], mx[:], start=False, stop=True)
            st["lg_ps"] = lg_ps

        def ach_b(b, st):
            acol = st_pool.tile([C, 1], F32, tag="acol")
            nc.scalar.activation(acol[:], st.pop("lg_ps")[:], AF.Sigmoid)
            ar_ps = s_psp.tile([1, C], F32, tag="s_ps")
            nc.tensor.transpose(ar_ps[:], acol[:], ident_t[:])
            arow = row_pool.tile([1, C], F32, tag="arow")
            nc.vector.tensor_copy(arow[:], ar_ps[:])
            st["arow"] = arow

        def ach_c(b, st):
            # bc reuses a v-pool slot: all v chunks of this batch are done by
            # the time ach_c runs, so no contention with the p2 pipeline
            bc_ps = v_psp.tile([C, C], F32, tag="v_ps")
            nc.tensor.matmul(bc_ps[:], ones_t[:], st.pop("arow")[:], start=True, stop=True)
            w2a = w2a_pool.tile([C, C], BF16, tag="w2a")
            nc.vector.tensor_tensor(w2a[:], w2t_t[:], bc_ps[:], ALU.mult)
            w2a1 = w2a_pool.tile([C, C], BF16, tag="w2a1")
            nc.vector.tensor_tensor(w2a1[:], w2a[:], wsc1_t[:], ALU.add)
            st["w2a"], st["w2a1"] = w2a, w2a1

        def ach(b, st):
            ach_a(b, st)
            ach_b(b, st)
            ach_c(b, st)

        tiles_per_stage = ost // OT

        def _out_dma(dst, src):
            eng = {"sync": nc.sync, "gpsimd": nc.gpsimd, "scalar": nc.scalar,
                   "vector": nc.vector}[out_eng]
            eng.dma_start(dst, src)

        def p3_out(b, st, i, o_ps):
            if i % tiles_per_stage == 0:
                st["ostage"] = out_pool.tile([C, ost], BF16, tag="ot", name="ostage")
            stage = st["ostage"]
            nc.scalar.activation(stage[:, (i % tiles_per_stage) * OT:
                                          (i % tiles_per_stage + 1) * OT],
                                 o_ps[:], AF.Relu, bias=t2_t[:], scale=1.0)
            if i % tiles_per_stage == tiles_per_stage - 1:
                g0 = (i // tiles_per_stage) * ost
                _out_dma(out_ap[b, :, g0:g0 + ost], stage[:])

        def p3_tile(b, st, i):
            xr, soft, w2a, w2a1 = st["xr"], st["soft"], st["w2a"], st["w2a1"]
            if debug and i == 0:
                nc.sync.dma_start(dbg_xr_ap[b], xr[:, 0:8])
                nc.sync.dma_start(dbg_sf_ap[b], soft[:, 0:8])
                nc.sync.dma_start(dbg_w2a_ap[b], w2a[:])
                nc.sync.dma_start(dbg_st_ap[b, :, 0:1], st["tpos"][:])
                nc.sync.dma_start(dbg_st_ap[b, :, 1:2], st["ssum_p"][:, 0:1])
                nc.sync.dma_start(dbg_st_ap[b, :, 2:3], st["smax_p"][:, 0:1])
            o_ps = o_psp.tile([C, OT], F32, tag="o_ps")
            b0 = i * OT
            nc.tensor.matmul(o_ps[:], wsc0_t[:], xr[:, b0:b0 + OT], start=True, stop=False)
            nc.tensor.matmul(o_ps[:], wsc2_t[:], xr[:, b0 + 2:b0 + 2 + OT], start=False, stop=False)
            nc.tensor.matmul(o_ps[:], w2a1[:], xr[:, b0 + 1:b0 + 1 + OT], start=False, stop=False)
            nc.tensor.matmul(o_ps[:], w2a[:], soft[:, b0:b0 + OT], start=False, stop=True)
            p3_out(b, st, i, o_ps)

        def p3_pair(b, st, pr):
            # two adjacent tiles share each weight load (ldweights amortized)
            xr, soft, w2a, w2a1 = st["xr"], st["soft"], st["w2a"], st["w2a1"]
            ia, ib = 2 * pr, 2 * pr + 1
            a0, b0 = ia * OT, ib * OT
            oa = o_psp.tile([C, OT], F32, tag="o_ps")
            ob = o_psp.tile([C, OT], F32, tag="o_ps")
            for w, da, sa in ((wsc0_t, 0, None), (wsc2_t, 2, None),
                              (w2a1, 1, None), (w2a, 0, soft)):
                first = w is wsc0_t
                last = sa is soft
                srcA = sa[:, a0:a0 + OT] if sa is not None else xr[:, a0 + da:a0 + da + OT]
                srcB = sa[:, b0:b0 + OT] if sa is not None else xr[:, b0 + da:b0 + da + OT]
                nc.tensor.matmul(oa[:], w[:], srcA, start=first, stop=last)
                nc.tensor.matmul(ob[:], w[:], srcB, start=first, stop=last)
            p3_out(b, st, ia, oa)
            p3_out(b, st, ib, ob)

        NP2 = L // PCH   # 8 p2 chunks
        NP3 = L // OT    # 16 p3 tiles

        p3pair = os.environ.get("K_P3PAIR", "0") == "1"

        def p3_with_interleave(jb, st_j, nxt):
            # p3 of batch j with batch j+1's p2 front-loaded (done well before
            # the tail) and its ach parts spread over the tail so the
            # cross-engine chain latency hides under p3 matmuls.
            if not p3pair:
                for i in range(NP3):
                    if nxt is not None and i < NP2:
                        p2_mm(*nxt, i)
                    p3_tile(jb, st_j, i)
                    if nxt is not None and i < NP2:
                        p2_ew(*nxt, i)
                    if nxt is not None:
                        if i == NP2 + 1:
                            ach_a(*nxt)
                        elif i == NP2 + 3:
                            ach_b(*nxt)
                        elif i == NP2 + 5:
                            ach_c(*nxt)
            else:
                for pr in range(NP3 // 2):
                    if nxt is not None and pr < NP2 // 2:
                        p2_mm(*nxt, 2 * pr)
                        p2_mm(*nxt, 2 * pr + 1)
                    p3_pair(jb, st_j, pr)
                    if nxt is not None and pr < NP2 // 2:
                        p2_ew(*nxt, 2 * pr)
                        p2_ew(*nxt, 2 * pr + 1)
                    if nxt is not None:
                        if pr == NP2 // 2:
                            ach_a(*nxt)
                        elif pr == NP2 // 2 + 1:
                            ach_b(*nxt)
                        elif pr == NP2 // 2 + 2:
                            ach_c(*nxt)

        loop_cm = tc.For_i(0, loop_reps, 1) if loop_reps else None
        if loop_cm is not None:
            loop_cm.__enter__()

        seq = [b for _ in range(reps) for b in range(NB)]
        states = {}
        # step s: dma(s) | mlp(s-1) | p3(s-2) + [p2+ach](s-1) interleaved | abs(s)
        for s in range(len(seq) + 2):
            if s < len(seq):
                states[s] = {}
                p1_dma(seq[s], states[s])
            if 1 <= s <= len(seq):
                j = s - 1
                mlp(seq[j], states[j])
            if 2 <= s:
                j = s - 2
                nxt = (seq[j + 1], states[j + 1]) if j + 1 < len(seq) else None
                p3_with_interleave(seq[j], states[j], nxt)
                del states[j]
            elif s == 1:
                # prologue: run p2+ach of batch 0 without a p3 to interleave into
                for p in range(NP2):
                    p2_mm(seq[0], states[0], p)
                    p2_ew(seq[0], states[0], p)
                ach(seq[0], states[0])
            if s < len(seq):
                p1_abs(seq[s], states[s])

        if loop_cm is not None:
            loop_cm.__exit__(None, None, None)

        # End-of-kernel fence: force waits on the tail of the pipeline so the
        # NEFF cannot signal completion while last-batch work / output DMAs
        # are still in flight. Allocating through the out_pool ring forces
        # WAR waits on the final output-DMA completion semaphores.
        for _k in range(3):
            dfence = out_pool.tile([C, ost], BF16, tag="ot", name="dfence")
            nc.vector.memset(dfence[:, 0:1], 0.0)
        nc.sync.drain()
        nc.gpsimd.drain()
        nc.scalar.drain()
        nc.vector.drain()

    nc.compile()
    _BUILD_CACHE[key] = nc
    return nc


def _host_weights(w_fc1, b_fc1, bn1_g, bn1_b, bn1_rm, bn1_rv, w_fc2, b_fc2,
                  w1, w2, w_sp, w_sc, bn2_g, bn2_b, bn2_rm, bn2_rv):
    f = np.float32
    bf = ml_dtypes.bfloat16
    s1 = (bn1_g / np.sqrt(bn1_rv + EPS)).astype(f)
    t1 = (bn1_b - bn1_rm * s1).astype(f)
    wfc1 = np.ascontiguousarray(((w_fc1 * s1[:, None]) / L).T, dtype=f)      # [C, CQ]
    b1e = np.ascontiguousarray((b_fc1 * s1 + t1)[:, None], dtype=f)          # [CQ, 1]
    wfc2 = np.ascontiguousarray(w_fc2.T, dtype=f)                            # [CQ, C]
    b2 = np.ascontiguousarray(b_fc2[:, None], dtype=f)                       # [C, 1]
    w_v = np.ascontiguousarray(w1[:, :, 0].T, dtype=bf)                      # [C, C]
    w2t = np.ascontiguousarray(w2[:, :, 0].T, dtype=bf)
    s2 = (bn2_g / np.sqrt(bn2_rv + EPS)).astype(f)
    t2 = np.ascontiguousarray((bn2_b - bn2_rm * s2)[:, None], dtype=f)
    wsc = [np.ascontiguousarray((w_sc[:, :, k] * s2[:, None]).T, dtype=bf)
           for k in range(3)]
    # banded matrices for the channel-axis conv of [mean, max] rows
    wm = (w_sp[0, 0, :] / L).astype(f)
    wx = w_sp[0, 1, :].astype(f)
    am = (wm[0] * np.eye(C, k=-1) + wm[1] * np.eye(C) + wm[2] * np.eye(C, k=1)).astype(f)
    ax = (wx[0] * np.eye(C, k=-1) + wx[1] * np.eye(C) + wx[2] * np.eye(C, k=1)).astype(f)
    ident = np.eye(C, dtype=f)
    return {
        "w_v": w_v, "wsc0": wsc[0], "wsc1": wsc[1], "wsc2": wsc[2],
        "w2t": w2t, "wfc1": wfc1, "b1e": b1e, "wfc2": wfc2, "b2": b2,
        "t2": t2, "ident": ident,
        "wam": np.ascontiguousarray(am.T), "wax": np.ascontiguousarray(ax.T),
    }


def _cast_x(x):
    return np.asarray(x).astype(ml_dtypes.bfloat16)


def kernel(x, w_fc1, b_fc1, bn1_g, bn1_b, bn1_rm, bn1_rv, w_fc2, b_fc2,
           w1, w2, w_sp, w_sc, bn2_g, bn2_b, bn2_rm, bn2_rv):
    x = np.asarray(x, dtype=np.float32)
    wd = _host_weights(np.asarray(w_fc1, np.float32), np.asarray(b_fc1, np.float32),
                       np.asarray(bn1_g, np.float32), np.asarray(bn1_b, np.float32),
                       np.asarray(bn1_rm, np.float32), np.asarray(bn1_rv, np.float32),
                       np.asarray(w_fc2, np.float32), np.asarray(b_fc2, np.float32),
                       np.asarray(w1, np.float32), np.asarray(w2, np.float32),
                       np.asarray(w_sp, np.float32), np.asarray(w_sc, np.float32),
                       np.asarray(bn2_g, np.float32), np.asarray(bn2_b, np.float32),
                       np.asarray(bn2_rm, np.float32), np.asarray(bn2_rv, np.float32))

    x_bf = x.astype(ml_dtypes.bfloat16)
    nc = _build()
    in_maps = []
    for c in range(N_CORES):
        m = dict(wd)
        m["x_dram"] = np.ascontiguousarray(x_bf[c * NB:(c + 1) * NB])
        in_maps.append(m)
    res = bass_utils.run_bass_kernel_spmd(nc, in_maps, core_ids=list(range(N_CORES)))
    out = np.concatenate([res.results[c]["out_dram"] for c in range(N_CORES)], axis=0)
    return out.astype(np.float32)
